# revision 1
# baseline (speedup 1.0000x reference)
"""Multi-head attention (B=2, S=4096, D=768, H=12) on 8 Trainium2 cores.

Sharding: core c handles batch c//4 and heads 3*(c%4)..3*(c%4)+2.
Each core computes its 3 heads end-to-end (QKV projection, causal
attention, its partial of the output projection); the host sums the 4
per-batch partials and adds the output bias.

Device algorithm per core (S=4096, DK=64, 3 heads), bf16 matmuls with
fp32 PSUM accumulation:
  - Q^T,K^T [e,s] via matmul chains (contract d=768), heads 0,1 packed on
    partitions 0-63/64-127 of slot 0, head 2 on partitions 0-63 of slot 1.
  - V [s,e] per head with a ones column appended (65 wide).
  - scores^T tiles [128k, 512q] per (key-block, query-group).
    Only causally-live key blocks are computed; diagonal blocks get exp()
    then a 0/1 mask multiply (exact zeros).
  - ctx^T + softmax denominator in one accumulating matmul:
    lhsT=[V|1] (65 cols) -> psum rows 0-63 ctx^T, row 64 = sum(exp).
  - denominator reciprocal broadcast across partitions via a K=1 matmul
    with a ones row, then one DVE multiply normalizes into bf16 ctx^T.
  - output projection from ctx^T (bf16) against wo^T slices.
"""

import sys

sys.path.insert(0, "/opt/trn_rl_repo")

import ml_dtypes
import numpy as np

import concourse.bass as bass
import concourse.mybir as mybir
import concourse.tile as tile
from concourse.bass_utils import run_bass_kernel_spmd

B, S, D, H = 2, 4096, 768, 12
DK = D // H          # 64
NCORES = 8
HPC = 3              # heads per core
E = HPC * DK         # 192 = per-core projection width
P = 128
DC = D // P          # 6 contraction chunks of 128
SG = S // 512        # 8 query groups of 512
SC = S // P          # 32 token chunks of 128
F32 = mybir.dt.float32
F32R = mybir.dt.float32r
BF16 = mybir.dt.bfloat16
EXP = mybir.ActivationFunctionType.Exp
IDENT = mybir.ActivationFunctionType.Identity
BF = ml_dtypes.bfloat16


def _split_multi_waits(nc):
    """This walrus build encodes exactly one sync wait per TPB instruction
    and refuses to split multi-wait instructions itself. Rewrite each block
    so extra waits land on same-engine NOPs directly before the owner."""
    k = 0
    for f in nc.m.functions:
        for blk in f.blocks:
            out = []
            changed = False
            for inst in blk.instructions:
                si = inst.sync_info
                if si is not None and len(si.on_wait) > 1:
                    changed = True
                    waits = list(si.on_wait)
                    for w in waits[:-1]:
                        nop = mybir.InstNoOp(name=f"splitw-{k}", ins=[], outs=[])
                        k += 1
                        nop.engine = inst.engine
                        nop.sync_info = mybir.SyncInfo(on_wait=[w], on_update=[])
                        out.append(nop)
                    inst.sync_info = mybir.SyncInfo(
                        on_wait=[waits[-1]], on_update=list(si.on_update)
                    )
                out.append(inst)
            if changed:
                blk.instructions = out


def _r(ap):
    return ap.bitcast(F32R)


def _build_program(repeat=1, parts="all"):
    nc = bass.Bass("TRN2", target_bir_lowering=False, debug=False)

    qT = nc.declare_dram_parameter("qT", [D, S], BF16, isOutput=False)
    kT = nc.declare_dram_parameter("kT", [D, S], BF16, isOutput=False)
    vT = nc.declare_dram_parameter("vT", [D, S], BF16, isOutput=False)
    wqT = nc.declare_dram_parameter("wqT", [D, E], BF16, isOutput=False)
    wkT = nc.declare_dram_parameter("wkT", [D, E], BF16, isOutput=False)
    wvT = nc.declare_dram_parameter("wvT", [D, 256], BF16, isOutput=False)
    woT = nc.declare_dram_parameter("woT", [E, D], BF16, isOutput=False)
    bq = nc.declare_dram_parameter("bq", [P, 2], F32, isOutput=False)  # (bq)/8 packed
    bk = nc.declare_dram_parameter("bk", [P, 2], F32, isOutput=False)
    bv = nc.declare_dram_parameter("bv", [P, 256], F32, isOutput=False)
    maskc = nc.declare_dram_parameter("maskc", [P, 4 * 512], BF16, isOutput=False)
    out_p = nc.declare_dram_parameter("out_p", [S, D], BF16, isOutput=True)

    qT_r = qT[:].rearrange("(dc p) s -> p dc s", p=P)
    kT_r = kT[:].rearrange("(dc p) s -> p dc s", p=P)
    vT_r = vT[:].rearrange("(dc p) s -> p dc s", p=P)

    rp = repeat if parts in ("all", "proj") else 1
    ra = repeat if parts in ("all", "attn") else 1

    with tile.TileContext(nc) as tc:
        import contextlib

        with contextlib.ExitStack() as ctx:
            const = ctx.enter_context(tc.tile_pool(name="const", bufs=1))
            persist = ctx.enter_context(tc.tile_pool(name="persist", bufs=1))

            # ---- constants ----
            wq_sb = const.tile([P, DC, E], BF16)
            nc.sync.dma_start(wq_sb[:], wqT[:].rearrange("(dc p) e -> p dc e", p=P))
            wk_sb = const.tile([P, DC, E], BF16)
            nc.sync.dma_start(wk_sb[:], wkT[:].rearrange("(dc p) e -> p dc e", p=P))
            wv_sb = const.tile([P, DC, 256], BF16)
            nc.sync.dma_start(wv_sb[:], wvT[:].rearrange("(dc p) e -> p dc e", p=P))
            wo_sb = const.tile([64, HPC, D], BF16)
            nc.sync.dma_start(wo_sb[:], woT[:].rearrange("(h p) o -> p h o", p=64))
            bq_sb = const.tile([P, 2], F32)
            nc.sync.dma_start(bq_sb[:], bq[:])
            bk_sb = const.tile([P, 2], F32)
            nc.sync.dma_start(bk_sb[:], bk[:])
            bv_sb = const.tile([P, 256], F32)
            nc.sync.dma_start(bv_sb[:], bv[:])
            mask_sb = const.tile([P, 4 * 512], BF16)
            nc.sync.dma_start(mask_sb[:], maskc[:])
            onesf = const.tile([P, 64], F32)
            nc.any.memset(onesf[:], 1.0)
            ones_sb = const.tile([P, 64], F32R)
            nc.vector.tensor_copy(ones_sb[:], onesf[:])
            onesb = const.tile([P, 64], BF16)
            nc.vector.tensor_copy(onesb[:], onesf[:])

            # ---- persistent activations ----
            # slot 0: heads 0 (parts 0-63) & 1 (parts 64-127); slot 1: head 2 low.
            QT_sb = persist.tile([P, 2, S], BF16)
            KT_sb = persist.tile([P, 2, S], BF16)
            V_sb = [
                persist.tile([P, SC, 65], BF16, tag=f"v{h}", name=f"v{h}")
                for h in range(HPC)
            ]
            ctxT_sb = persist.tile([64, HPC, S], BF16)

            for h in range(HPC):
                nc.vector.tensor_copy(V_sb[h][:, :, 64], onesb[:, 0:SC])

            # (slot, base partition, rows) for each head's Q/K storage
            head_loc = [(0, 0, 64), (0, 64, 64), (1, 0, 64)]

            # ---- phase 1: Q/K projections;  phase 2: V projection ----
            for _rp in range(rp):
                with tc.tile_pool(name="pj", bufs=4) as pj, \
                     tc.tile_pool(name="pjp", bufs=4, space="PSUM") as pjp:
                    for (w_sb, b_sb, dst, scale) in (
                        (wq_sb, bq_sb, QT_sb, 0.125),
                        (wk_sb, bk_sb, KT_sb, 1.0),
                    ):
                        src = qT_r if dst is QT_sb else kT_r
                        for sg2 in range(SG // 2):
                            stg = pj.tile([P, DC, 1024], BF16, tag="stage")
                            nc.sync.dma_start(
                                stg[:], src[:, :, sg2 * 1024:(sg2 + 1) * 1024]
                            )
                            for half in range(2):
                                sg = 2 * sg2 + half
                                for ec, em in ((0, P), (1, 64)):
                                    ps = pjp.tile([P, 512], F32, tag="qk")
                                    for dc in range(DC):
                                        nc.tensor.matmul(
                                            ps[:em, :],
                                            w_sb[:, dc, ec * P:ec * P + em],
                                            stg[:, dc, half * 512:(half + 1) * 512],
                                            start=(dc == 0), stop=(dc == DC - 1),
                                        )
                                    nc.scalar.activation(
                                        dst[0:em, ec, sg * 512:(sg + 1) * 512]
                                        if ec == 1
                                        else dst[:, 0, sg * 512:(sg + 1) * 512],
                                        ps[:em, :],
                                        IDENT,
                                        bias=b_sb[:em, ec:ec + 1],
                                        scale=scale,
                                    )

                    for sc4 in range(SC // 4):
                        vstg = pj.tile([P, DC, 512], BF16, tag="vstage")
                        nc.sync.dma_start(
                            vstg[:], vT_r[:, :, sc4 * 512:(sc4 + 1) * 512]
                        )
                        for quart in range(4):
                            sc = 4 * sc4 + quart
                            ps = pjp.tile([P, 256], F32, tag="v")
                            for dc in range(DC):
                                nc.tensor.matmul(
                                    ps[:],
                                    vstg[:, dc, quart * P:(quart + 1) * P],
                                    wv_sb[:, dc, :],
                                    start=(dc == 0), stop=(dc == DC - 1),
                                )
                            for h in range(HPC):
                                nc.vector.tensor_tensor(
                                    V_sb[h][:, sc, 0:64],
                                    ps[:, h * 64:(h + 1) * 64],
                                    bv_sb[:, h * 64:(h + 1) * 64],
                                    mybir.AluOpType.add,
                                )

            # ---- phase 3: attention;  phase 4: output projection ----
            for _ra in range(ra):
                with tc.tile_pool(name="att", bufs=8) as att, \
                     tc.tile_pool(name="nrm", bufs=3) as nrm, \
                     tc.tile_pool(name="stp", bufs=4, space="PSUM") as stp, \
                     tc.tile_pool(name="ctxp", bufs=3, space="PSUM") as ctxp, \
                     tc.tile_pool(name="bcp", bufs=1, space="PSUM") as bcp:
                    for qg in range(SG):
                        nkb = 4 * (qg + 1)
                        ctx_ps = {}
                        for h in range(HPC):
                            ctx_ps[h] = ctxp.tile(
                                [P, 512], F32, tag="ctx", name=f"ctx{h}"
                            )
                        # one [128,512] score tile per key-block: 4 PSUM slots
                        # in flight so no head ever stalls on exp draining.
                        for kb in range(nkb):
                            # all 3 heads' score matmuls back-to-back: heads
                            # 0/1 use PE row groups 0-63/64-127 and execute
                            # concurrently; 4 PSUM slots so none stalls on exp.
                            sts = {}
                            for h in range(HPC):
                                slot, p0, rows = head_loc[h]
                                sts[h] = stp.tile([P, 512], F32, tag="st",
                                                  name=f"st{h}")
                                nc.tensor.matmul(
                                    sts[h][:],
                                    KT_sb[p0:p0 + rows, slot, kb * P:(kb + 1) * P],
                                    QT_sb[p0:p0 + rows, slot,
                                          qg * 512:(qg + 1) * 512],
                                    start=True, stop=True,
                                )
                            ets = {}
                            for h in range(HPC):
                                et = att.tile([P, 512], BF16, tag="et")
                                if kb >= nkb - 4:
                                    # diagonal: exp then 0/1 mask multiply
                                    ete = att.tile([P, 512], BF16, tag="ete")
                                    nc.scalar.activation(ete[:], sts[h][:], EXP)
                                    j = kb - (nkb - 4)
                                    nc.vector.tensor_tensor(
                                        et[:], ete[:],
                                        mask_sb[:, j * 512:(j + 1) * 512],
                                        mybir.AluOpType.mult,
                                    )
                                else:
                                    nc.scalar.activation(et[:], sts[h][:], EXP)
                                ets[h] = et
                            for h in range(HPC):
                                nc.tensor.matmul(
                                    ctx_ps[h][0:65, :],
                                    V_sb[h][:, kb, :],
                                    ets[h][:],
                                    start=(kb == 0), stop=(kb == nkb - 1),
                                )
                        for h in range(HPC):
                            rc = nrm.tile([P, 512], F32R, tag="rc")
                            with nc.allow_low_precision(
                                reason="softmax denominator reciprocal; f32r "
                                "rounding is benign here"
                            ):
                                nc.vector.reciprocal(
                                    rc[64:65, :], ctx_ps[h][64:65, :]
                                )
                            bc = bcp.tile([64, 512], F32, tag="bc")
                            nc.tensor.matmul(
                                bc[:], _r(ones_sb[64:65, :]), rc[64:65, :],
                                start=True, stop=True,
                            )
                            rcb = nrm.tile([64, 512], F32, tag="rcb")
                            nc.vector.tensor_copy(rcb[:], bc[:])
                            nc.vector.tensor_tensor(
                                ctxT_sb[0:64, h, qg * 512:(qg + 1) * 512],
                                ctx_ps[h][0:64, :],
                                rcb[:],
                                mybir.AluOpType.mult,
                            )

                with tc.tile_pool(name="ob", bufs=3) as ob, \
                     tc.tile_pool(name="op", bufs=2, space="PSUM") as op:
                    for sc in range(SC):
                        osb = ob.tile([P, D], BF16, tag="osb")
                        for og, o0, ow in ((0, 0, 512), (1, 512, 256)):
                            ps = op.tile([P, 512], F32, tag=f"og{og}")
                            for h in range(HPC):
                                nc.tensor.matmul(
                                    ps[:, :ow],
                                    ctxT_sb[0:64, h, sc * P:(sc + 1) * P],
                                    wo_sb[:, h, o0:o0 + ow],
                                    start=(h == 0), stop=(h == HPC - 1),
                                )
                            if og == 0:
                                nc.vector.tensor_copy(osb[:, o0:o0 + ow], ps[:, :ow])
                            else:
                                nc.scalar.copy(osb[:, o0:o0 + ow], ps[:, :ow])
                        nc.sync.dma_start(out_p[sc * P:(sc + 1) * P, :], osb[:])

    _split_multi_waits(nc)
    return nc


_CACHED_NC = None


def _get_nc():
    global _CACHED_NC
    if _CACHED_NC is None:
        _CACHED_NC = _build_program()
    return _CACHED_NC


def _numpy_reference(q, k, v, wq, bq, wk, bk, wv, bv, wo, bo, mask):
    """Fallback for masks the fast path does not handle (non-causal)."""
    out = np.empty((B, S, D), np.float32)
    scale = 1.0 / np.sqrt(DK)
    for b in range(B):
        Q = (q[b] @ wq.T + bq).reshape(S, H, DK).transpose(1, 0, 2)
        K = (k[b] @ wk.T + bk).reshape(S, H, DK).transpose(1, 0, 2)
        V = (v[b] @ wv.T + bv).reshape(S, H, DK).transpose(1, 0, 2)
        ctx = np.empty((H, S, DK), np.float32)
        for h in range(H):
            s = (Q[h] @ K[h].T) * scale
            s = np.where(mask, s, -1e9)
            s -= s.max(axis=-1, keepdims=True)
            e = np.exp(s)
            p = e / e.sum(axis=-1, keepdims=True)
            ctx[h] = p @ V[h]
        out[b] = ctx.transpose(1, 0, 2).reshape(S, D) @ wo.T + bo
    return out


def _prepare_in_maps(q, k, v, wq, bq, wk, bk, wv, bv, wo):
    # causal 0/1 diagonal-block masks: maskc[k, j*512+q] = (128j + k) <= q
    kk = np.arange(P)[:, None]
    qq = np.arange(512)[None, :]
    maskc = np.zeros((P, 4, 512), np.float32)
    for j in range(4):
        maskc[:, j, :] = (P * j + kk) <= qq
    maskc = np.ascontiguousarray(maskc.reshape(P, 4 * 512)).astype(BF)

    wqT = np.ascontiguousarray(wq.T).astype(BF)  # [d_in, e_out]
    wkT = np.ascontiguousarray(wk.T).astype(BF)
    wvT = np.ascontiguousarray(wv.T).astype(BF)
    woT = np.ascontiguousarray(wo.T)             # [e_in, d_out]

    qTb = [np.ascontiguousarray(q[b].T).astype(BF) for b in range(B)]
    kTb = [np.ascontiguousarray(k[b].T).astype(BF) for b in range(B)]
    vTb = [np.ascontiguousarray(v[b].T).astype(BF) for b in range(B)]

    def pack_bias(bvec, scale):
        t = np.zeros((P, 2), np.float32)
        t[:, 0] = bvec[:P] * scale
        t[:64, 1] = bvec[P:E] * scale
        return t

    in_maps = []
    for c in range(NCORES):
        b = c // 4
        e0 = 3 * (c % 4) * DK
        wvp = np.zeros((D, 256), BF)
        wvp[:, :E] = wvT[:, e0:e0 + E]
        bvp = np.zeros((P, 256), np.float32)
        bvp[:, :E] = bv[e0:e0 + E][None, :]
        in_maps.append({
            "qT": qTb[b],
            "kT": kTb[b],
            "vT": vTb[b],
            "wqT": np.ascontiguousarray(wqT[:, e0:e0 + E]),
            "wkT": np.ascontiguousarray(wkT[:, e0:e0 + E]),
            "wvT": wvp,
            "woT": np.ascontiguousarray(woT[e0:e0 + E, :]).astype(BF),
            "bq": pack_bias(bq[e0:e0 + E], 0.125),
            "bk": pack_bias(bk[e0:e0 + E], 1.0),
            "bv": bvp,
            "maskc": maskc,
        })
    return in_maps


def kernel(q, k, v, wq, bq, wk, bk, wv, bv, wo, bo, mask, **_unused):
    q = np.asarray(q, np.float32)
    k = np.asarray(k, np.float32)
    v = np.asarray(v, np.float32)
    wq = np.asarray(wq, np.float32)
    wk = np.asarray(wk, np.float32)
    wv = np.asarray(wv, np.float32)
    wo = np.asarray(wo, np.float32)
    bq = np.asarray(bq, np.float32)
    bk = np.asarray(bk, np.float32)
    bv = np.asarray(bv, np.float32)
    bo = np.asarray(bo, np.float32)
    mask = np.asarray(mask)

    tril = np.tril(np.ones((S, S), bool))
    if mask.shape != (S, S) or not np.array_equal(mask.astype(bool), tril):
        return _numpy_reference(q, k, v, wq, bq, wk, bk, wv, bv, wo, bo, mask)

    in_maps = _prepare_in_maps(q, k, v, wq, bq, wk, bk, wv, bv, wo)
    nc = _get_nc()
    res = run_bass_kernel_spmd(nc, in_maps, core_ids=list(range(NCORES)))

    out = np.empty((B, S, D), np.float32)
    for b in range(B):
        acc = res.results[4 * b]["out_p"].astype(np.float32)
        for c in range(4 * b + 1, 4 * b + 4):
            acc = acc + res.results[c]["out_p"].astype(np.float32)
        out[b] = acc + bo[None, :]
    return out



# revision 2
# speedup vs baseline: 3.0104x; 3.0104x over previous
"""Multi-head attention (B=2, S=4096, D=768, H=12) on 8 Trainium2 cores.

Sharding: core c handles batch c//4 and heads 3*(c%4)..3*(c%4)+2.
Each core computes its 3 heads end-to-end (QKV projection, causal
attention, its partial of the output projection); the host sums the 4
per-batch partials and adds the output bias.

v2: fully software-pipelined single schedule. The projection DMAs +
matmuls, attention (per query-group), and output projection are
interleaved so that the DMA-bound projection prologue and the PE-bound
output projection hide under the ACT(exp)-bound attention window:
  qk0 v0 a0 o0 v1 a1 o1 qk1 v2 a2 o2 v3 a3 o3 qk2 v4 ... a7 o7
Engine assignment: ACT runs nothing but Exp (no act-table reloads);
projection epilogues, broadcast, og1 copies and out DMAs run on the
otherwise idle Pool/gpsimd engine; mask mults, reciprocals, ctx
normalization and og0 copies on DVE. Diagonal score/exp/ctx tiles are
trimmed to the causally live query range.
"""

import sys

sys.path.insert(0, "/opt/trn_rl_repo")

import ml_dtypes
import numpy as np

import concourse.bass as bass
import concourse.mybir as mybir
import concourse.tile as tile
from concourse.bass_utils import run_bass_kernel_spmd

B, S, D, H = 2, 4096, 768, 12
DK = D // H          # 64
NCORES = 8
HPC = 3              # heads per core
E = HPC * DK         # 192 = per-core projection width
P = 128
DC = D // P          # 6 contraction chunks of 128
SG = S // 512        # 8 query groups of 512
SC = S // P          # 32 token chunks of 128
F32 = mybir.dt.float32
F32R = mybir.dt.float32r
BF16 = mybir.dt.bfloat16
EXP = mybir.ActivationFunctionType.Exp
BF = ml_dtypes.bfloat16
MUL = mybir.AluOpType.mult
ADD = mybir.AluOpType.add

# (slot, base partition) for each head's Q/K storage
HEAD_LOC = [(0, 0), (0, 64), (1, 0)]


def _split_multi_waits(nc):
    """This walrus build encodes exactly one sync wait per TPB instruction
    and refuses to split multi-wait instructions itself. Rewrite each block
    so extra waits land on same-engine NOPs directly before the owner."""
    k = 0
    for f in nc.m.functions:
        for blk in f.blocks:
            out = []
            changed = False
            for inst in blk.instructions:
                si = inst.sync_info
                if si is not None and len(si.on_wait) > 1:
                    changed = True
                    waits = list(si.on_wait)
                    for w in waits[:-1]:
                        nop = mybir.InstNoOp(name=f"splitw-{k}", ins=[], outs=[])
                        k += 1
                        nop.engine = inst.engine
                        nop.sync_info = mybir.SyncInfo(on_wait=[w], on_update=[])
                        out.append(nop)
                    inst.sync_info = mybir.SyncInfo(
                        on_wait=[waits[-1]], on_update=list(si.on_update)
                    )
                out.append(inst)
            if changed:
                blk.instructions = out


def _build_program(repeat=1, parts="all"):
    nc = bass.Bass("TRN2", target_bir_lowering=False, debug=False)

    qT = nc.declare_dram_parameter("qT", [D, S], BF16, isOutput=False)
    kT = nc.declare_dram_parameter("kT", [D, S], BF16, isOutput=False)
    vT = nc.declare_dram_parameter("vT", [D, S], BF16, isOutput=False)
    wqT = nc.declare_dram_parameter("wqT", [D, E], BF16, isOutput=False)
    wkT = nc.declare_dram_parameter("wkT", [D, E], BF16, isOutput=False)
    wvT = nc.declare_dram_parameter("wvT", [D, E], BF16, isOutput=False)
    woT = nc.declare_dram_parameter("woT", [E, D], BF16, isOutput=False)
    bq = nc.declare_dram_parameter("bq", [P, 2], F32, isOutput=False)  # *0.125
    bk = nc.declare_dram_parameter("bk", [P, 2], F32, isOutput=False)
    bv = nc.declare_dram_parameter("bv", [P, E], F32, isOutput=False)
    maskc = nc.declare_dram_parameter("maskc", [P, 4 * 512], BF16, isOutput=False)
    out_p = nc.declare_dram_parameter("out_p", [S, D], BF16, isOutput=True)

    qT_r = qT[:].rearrange("(dc p) s -> p dc s", p=P)
    kT_r = kT[:].rearrange("(dc p) s -> p dc s", p=P)
    vT_r = vT[:].rearrange("(dc p) s -> p dc s", p=P)

    with tile.TileContext(nc) as tc:
        import contextlib

        with contextlib.ExitStack() as ctx:
            const = ctx.enter_context(tc.tile_pool(name="const", bufs=1))
            persist = ctx.enter_context(tc.tile_pool(name="persist", bufs=1))

            # ---- constants (criticality order: QK weights first, wo last) ----
            wq_sb = const.tile([P, DC, E], BF16)
            nc.sync.dma_start(wq_sb[:], wqT[:].rearrange("(dc p) e -> p dc e", p=P))
            wk_sb = const.tile([P, DC, E], BF16)
            nc.sync.dma_start(wk_sb[:], wkT[:].rearrange("(dc p) e -> p dc e", p=P))
            wv_sb = const.tile([P, DC, E], BF16)
            nc.sync.dma_start(wv_sb[:], wvT[:].rearrange("(dc p) e -> p dc e", p=P))
            bq_sb = const.tile([P, 2], F32)
            nc.sync.dma_start(bq_sb[:], bq[:])
            bk_sb = const.tile([P, 2], F32)
            nc.sync.dma_start(bk_sb[:], bk[:])
            bv_sb = const.tile([P, E], F32)
            nc.sync.dma_start(bv_sb[:], bv[:])
            mask_sb = const.tile([P, 4 * 512], BF16)
            nc.sync.dma_start(mask_sb[:], maskc[:])
            wo_sb = const.tile([64, HPC, D], BF16)
            nc.sync.dma_start(wo_sb[:], woT[:].rearrange("(h p) o -> p h o", p=64))
            onesb = const.tile([P, SC], BF16)
            nc.any.memset(onesb[:], 1.0)
            onesf = const.tile([P, 64], F32)
            nc.any.memset(onesf[:], 1.0)
            ones_sb = const.tile([P, 64], F32R)
            nc.vector.tensor_copy(ones_sb[:], onesf[:])
            # warm the Exp activation table while the staging DMAs run
            warm = const.tile([1, 1], F32)
            nc.scalar.activation(warm[:], onesb[0:1, 0:1], EXP)

            # ---- persistent activations ----
            QT_sb = persist.tile([P, 2, S], BF16)
            KT_sb = persist.tile([P, 2, S], BF16)
            V_sb = persist.tile([P, SC, HPC, 65], BF16)
            ctxT_sb = persist.tile([64, HPC, S], BF16)

            for h in range(HPC):
                nc.vector.tensor_copy(V_sb[:, :, h, 64], onesb[:])

            for _rep in range(repeat):
                with tc.tile_pool(name="stg", bufs=3) as stg_pool, \
                     tc.tile_pool(name="vstg", bufs=2) as vstg_pool, \
                     tc.tile_pool(name="att", bufs=6) as att, \
                     tc.tile_pool(name="att2", bufs=2) as att2, \
                     tc.tile_pool(name="nrm", bufs=4) as nrm, \
                     tc.tile_pool(name="ob", bufs=2) as ob, \
                     tc.tile_pool(name="pjp", bufs=2, space="PSUM") as pjp, \
                     tc.tile_pool(name="vp", bufs=1, space="PSUM") as vp, \
                     tc.tile_pool(name="stp", bufs=2, space="PSUM") as stp, \
                     tc.tile_pool(name="ctxp", bufs=3, space="PSUM") as ctxp:

                    qk_stages = {}   # ("q"|"k", sg) -> staged tile
                    v_stages = {}    # sc4 -> staged tile

                    def stage_qk(which, sg, eng=None):
                        src = qT_r if which == "q" else kT_r
                        stg = stg_pool.tile([P, DC, 512], BF16, tag="stage")
                        (eng or nc.sync).dma_start(
                            stg[:], src[:, :, sg * 512:(sg + 1) * 512]
                        )
                        qk_stages[(which, sg)] = stg

                    def proj_qk(which, sg, ec):
                        # one [em, 512] projection chain + epilogue
                        w_sb, b_sb, dst, scale = (
                            (wq_sb, bq_sb, QT_sb, 0.125)
                            if which == "q"
                            else (wk_sb, bk_sb, KT_sb, 1.0)
                        )
                        stg = qk_stages[(which, sg)]
                        em = P if ec == 0 else 64
                        ps = pjp.tile([P, 512], F32, tag="qk")
                        for dc in range(DC):
                            nc.tensor.matmul(
                                ps[:em, :],
                                w_sb[:, dc, ec * P:ec * P + em],
                                stg[:, dc, :],
                                start=(dc == 0), stop=(dc == DC - 1),
                            )
                        dst_ap = (
                            dst[0:64, 1, sg * 512:(sg + 1) * 512]
                            if ec == 1
                            else dst[:, 0, sg * 512:(sg + 1) * 512]
                        )
                        # GPSIMD cannot read PSUM on hw; DVE has the slack
                        nc.vector.tensor_scalar(
                            dst_ap, ps[:em, :],
                            scale, b_sb[:em, ec:ec + 1],
                            MUL, ADD,
                        )

                    def stage_v(j):
                        vstg = vstg_pool.tile([P, DC, 512], BF16, tag="vstage")
                        nc.sync.dma_start(
                            vstg[:], vT_r[:, :, j * 512:(j + 1) * 512]
                        )
                        v_stages[j] = vstg

                    def proj_v(sc):
                        vstg = v_stages[sc // 4]
                        quart = sc % 4
                        ps = vp.tile([P, 256], F32, tag="v")
                        for dc in range(DC):
                            nc.tensor.matmul(
                                ps[:, 0:E],
                                vstg[:, dc, quart * P:(quart + 1) * P],
                                wv_sb[:, dc, :],
                                start=(dc == 0), stop=(dc == DC - 1),
                            )
                        for h in range(HPC):
                            # tensor_tensor on Pool needs the standard ucode
                            # library, which partition_broadcast displaced;
                            # DVE has the slack.
                            nc.vector.tensor_tensor(
                                V_sb[:, sc, h, 0:64],
                                ps[:, h * 64:(h + 1) * 64],
                                bv_sb[:, h * 64:(h + 1) * 64],
                                ADD,
                            )

                    def emit_attn(g, fillers=(), n_pre=0):
                        # fillers: unit closures paced between kb steps so
                        # the PE queue never has a long proj-only run that
                        # would starve the exp stream. They are emitted
                        # between the exps and ctx matmuls of a kb step:
                        # the first n_pre fillers (prev group's normalize)
                        # must precede this group's first ctx matmul in PE
                        # program order, or the ctx-ring wait deadlocks the
                        # in-order scheduling pass.
                        fillers = list(fillers)
                        filled = 0
                        nkb = 4 * (g + 1)
                        ctx_ps = [
                            ctxp.tile([P, 512], F32, tag="ctx", name=f"ctx{h}")
                            for h in range(HPC)
                        ]
                        for kb in range(nkb):
                            dj = kb - (nkb - 4)  # diagonal index 0..3, else <0
                            qoff = 128 * dj if dj > 0 else 0
                            w = 512 - qoff
                            sts = {}
                            for h in range(HPC):
                                slot, p0 = HEAD_LOC[h]
                                st = stp.tile([P, 512], F32, tag="st",
                                              name=f"st{h}")
                                nc.tensor.matmul(
                                    st[:, 0:w],
                                    KT_sb[p0:p0 + 64, slot,
                                          kb * P:(kb + 1) * P],
                                    QT_sb[p0:p0 + 64, slot,
                                          g * 512 + qoff:(g + 1) * 512],
                                    start=True, stop=True,
                                )
                                sts[h] = st
                            ets = {}
                            for h in range(HPC):
                                et = att.tile([P, 512], BF16, tag="et")
                                if dj >= 0:
                                    # diagonal: exp then 0/1 mask multiply
                                    ete = att2.tile([P, 512], BF16, tag="ete")
                                    nc.scalar.activation(
                                        ete[:, 0:w], sts[h][:, 0:w], EXP
                                    )
                                    # SBUF-only, so it can ride the idle
                                    # Pool engine (GPSIMD cannot read PSUM)
                                    nc.gpsimd.tensor_tensor(
                                        et[:, 0:w], ete[:, 0:w],
                                        mask_sb[:, dj * 512 + qoff:
                                                (dj + 1) * 512],
                                        MUL,
                                    )
                                else:
                                    nc.scalar.activation(
                                        et[:, 0:w], sts[h][:, 0:w], EXP
                                    )
                                ets[h] = et
                            want = max(n_pre, (kb + 1) * len(fillers) // nkb)
                            while filled < min(want, len(fillers)):
                                fillers[filled]()
                                filled += 1
                            for h in range(HPC):
                                nc.tensor.matmul(
                                    ctx_ps[h][0:65, qoff:512],
                                    V_sb[:, kb, h, :],
                                    ets[h][:, 0:w],
                                    start=(kb == 0), stop=(kb == nkb - 1),
                                )
                        while filled < len(fillers):
                            fillers[filled]()
                            filled += 1
                        # reciprocals now: ctx chains just stopped, and the
                        # next group's normalize fillers need them without a
                        # DVE round-trip delay.
                        rcs = []
                        for h in range(HPC):
                            rc = nrm.tile([P, 512], F32R, tag="rc")
                            with nc.allow_low_precision(
                                reason="softmax denominator reciprocal; f32r "
                                "rounding is benign here"
                            ):
                                nc.vector.reciprocal(
                                    rc[64:65, :], ctx_ps[h][64:65, :]
                                )
                            rcs.append(rc)
                        return ctx_ps, rcs

                    def norm_h(g, ctx_ps, rcs, h):
                        # ctxT[h, qg slice] = ctx / denominator; denominator
                        # reciprocal broadcast across partitions 0-63 via a
                        # K=1 matmul with a ones row.
                        rc = rcs[h]
                        bc = stp.tile([P, 512], F32, tag="st")
                        nc.tensor.matmul(
                            bc[0:64, :],
                            ones_sb[64:65, :].bitcast(F32R),
                            rc[64:65, :],
                            start=True, stop=True,
                        )
                        rcb = nrm.tile([64, 512], F32, tag="rcb")
                        nc.vector.tensor_copy(rcb[:], bc[0:64, :])
                        nc.vector.tensor_tensor(
                            ctxT_sb[0:64, h, g * 512:(g + 1) * 512],
                            ctx_ps[h][0:64, :],
                            rcb[:],
                            MUL,
                        )

                    def oproj_sc(sc):
                        osb = ob.tile([P, D], BF16, tag="osb")
                        ps0 = pjp.tile([P, 512], F32, tag="qk")
                        for h in range(HPC):
                            nc.tensor.matmul(
                                ps0[:, :],
                                ctxT_sb[0:64, h, sc * P:(sc + 1) * P],
                                wo_sb[:, h, 0:512],
                                start=(h == 0), stop=(h == HPC - 1),
                            )
                        nc.vector.tensor_copy(osb[:, 0:512], ps0[:, :])
                        ps1 = vp.tile([P, 256], F32, tag="v")
                        for h in range(HPC):
                            nc.tensor.matmul(
                                ps1[:, :],
                                ctxT_sb[0:64, h, sc * P:(sc + 1) * P],
                                wo_sb[:, h, 512:768],
                                start=(h == 0), stop=(h == HPC - 1),
                            )
                        nc.vector.tensor_copy(osb[:, 512:768], ps1[:, :])
                        nc.sync.dma_start(
                            out_p[sc * P:(sc + 1) * P, :], osb[:]
                        )

                    def mk(fn, *a):
                        return lambda: fn(*a)

                    # --- software-pipelined schedule with paced fillers ---
                    # prerequisites for attn group 0; q0/k0 staged via the
                    # idle DVE/ACT queues so they overlap the const DMAs
                    # still draining on the sync queue.
                    stage_qk("q", 0, eng=nc.scalar)
                    stage_qk("k", 0, eng=nc.scalar)
                    stage_v(0)
                    proj_qk("q", 0, 0)
                    proj_qk("q", 0, 1)
                    proj_qk("k", 0, 0)
                    proj_qk("k", 0, 1)
                    for sc in range(4):
                        proj_v(sc)

                    prev = None  # (g, ctx_ps, rcs) of the previous group
                    for g in range(SG):
                        fill = []
                        if prev is not None:
                            # normalize the previous group first (unblocks
                            # its ctx psum ring + its output projection)
                            pg, pctx, prcs = prev
                            for h in range(HPC):
                                fill.append(mk(norm_h, pg, pctx, prcs, h))
                        gn = g + 1
                        if gn < SG:
                            # stage + project inputs needed by group g+1:
                            # QT[gn], KT[key blocks 4g+4..4g+7], V same rows.
                            fill.append(mk(stage_qk, "q", gn))
                            fill.append(mk(stage_qk, "k", gn))
                            fill.append(mk(stage_v, gn))
                            fill.append(mk(proj_qk, "q", gn, 0))
                            fill.append(mk(proj_qk, "k", gn, 0))
                            fill.append(mk(proj_v, 4 * gn))
                            fill.append(mk(proj_qk, "q", gn, 1))
                            fill.append(mk(proj_qk, "k", gn, 1))
                            fill.append(mk(proj_v, 4 * gn + 1))
                            fill.append(mk(proj_v, 4 * gn + 2))
                            fill.append(mk(proj_v, 4 * gn + 3))
                        # output projection, shifted 2 groups back so its PE
                        # load lands where ACT has surplus (late groups);
                        # group 6's rides with 5's in group 7.
                        opg = []
                        if g >= 2:
                            opg.append(g - 2)
                        if g == SG - 1:
                            opg.append(g - 1)
                        for pg2 in opg:
                            for sc in range(4 * pg2, 4 * pg2 + 4):
                                fill.append(mk(oproj_sc, sc))
                        ctx_ps, rcs = emit_attn(
                            g, fill, n_pre=HPC if prev is not None else 0
                        )
                        prev = (g, ctx_ps, rcs)
                    for h in range(HPC):
                        norm_h(SG - 1, prev[1], prev[2], h)
                    for sc in range(4 * (SG - 1), 4 * SG):
                        oproj_sc(sc)

    _split_multi_waits(nc)
    return nc


_CACHED_NC = None


def _get_nc():
    global _CACHED_NC
    if _CACHED_NC is None:
        _CACHED_NC = _build_program()
    return _CACHED_NC


def _numpy_reference(q, k, v, wq, bq, wk, bk, wv, bv, wo, bo, mask):
    """Fallback for masks the fast path does not handle (non-causal)."""
    out = np.empty((B, S, D), np.float32)
    scale = 1.0 / np.sqrt(DK)
    for b in range(B):
        Q = (q[b] @ wq.T + bq).reshape(S, H, DK).transpose(1, 0, 2)
        K = (k[b] @ wk.T + bk).reshape(S, H, DK).transpose(1, 0, 2)
        V = (v[b] @ wv.T + bv).reshape(S, H, DK).transpose(1, 0, 2)
        ctx = np.empty((H, S, DK), np.float32)
        for h in range(H):
            s = (Q[h] @ K[h].T) * scale
            s = np.where(mask, s, -1e9)
            s -= s.max(axis=-1, keepdims=True)
            e = np.exp(s)
            p = e / e.sum(axis=-1, keepdims=True)
            ctx[h] = p @ V[h]
        out[b] = ctx.transpose(1, 0, 2).reshape(S, D) @ wo.T + bo
    return out


def _prepare_in_maps(q, k, v, wq, bq, wk, bk, wv, bv, wo):
    # causal 0/1 diagonal-block masks: maskc[k, j*512+q] = (128j + k) <= q
    kk = np.arange(P)[:, None]
    qq = np.arange(512)[None, :]
    maskc = np.zeros((P, 4, 512), np.float32)
    for j in range(4):
        maskc[:, j, :] = (P * j + kk) <= qq
    maskc = np.ascontiguousarray(maskc.reshape(P, 4 * 512)).astype(BF)

    wqT = np.ascontiguousarray(wq.T).astype(BF)  # [d_in, e_out]
    wkT = np.ascontiguousarray(wk.T).astype(BF)
    wvT = np.ascontiguousarray(wv.T).astype(BF)
    woT = np.ascontiguousarray(wo.T)             # [e_in, d_out]

    qTb = [np.ascontiguousarray(q[b].T).astype(BF) for b in range(B)]
    kTb = [np.ascontiguousarray(k[b].T).astype(BF) for b in range(B)]
    vTb = [np.ascontiguousarray(v[b].T).astype(BF) for b in range(B)]

    def pack_bias(bvec, scale):
        t = np.zeros((P, 2), np.float32)
        t[:, 0] = bvec[:P] * scale
        t[:64, 1] = bvec[P:E] * scale
        return t

    in_maps = []
    for c in range(NCORES):
        b = c // 4
        e0 = 3 * (c % 4) * DK
        bvp = np.broadcast_to(bv[e0:e0 + E][None, :], (P, E)).copy()
        in_maps.append({
            "qT": qTb[b],
            "kT": kTb[b],
            "vT": vTb[b],
            "wqT": np.ascontiguousarray(wqT[:, e0:e0 + E]),
            "wkT": np.ascontiguousarray(wkT[:, e0:e0 + E]),
            "wvT": np.ascontiguousarray(wvT[:, e0:e0 + E]),
            "woT": np.ascontiguousarray(woT[e0:e0 + E, :]).astype(BF),
            "bq": pack_bias(bq[e0:e0 + E], 0.125),
            "bk": pack_bias(bk[e0:e0 + E], 1.0),
            "bv": bvp,
            "maskc": maskc,
        })
    return in_maps


def kernel(q, k, v, wq, bq, wk, bk, wv, bv, wo, bo, mask, **_unused):
    q = np.asarray(q, np.float32)
    k = np.asarray(k, np.float32)
    v = np.asarray(v, np.float32)
    wq = np.asarray(wq, np.float32)
    wk = np.asarray(wk, np.float32)
    wv = np.asarray(wv, np.float32)
    wo = np.asarray(wo, np.float32)
    bq = np.asarray(bq, np.float32)
    bk = np.asarray(bk, np.float32)
    bv = np.asarray(bv, np.float32)
    bo = np.asarray(bo, np.float32)
    mask = np.asarray(mask)

    tril = np.tril(np.ones((S, S), bool))
    if mask.shape != (S, S) or not np.array_equal(mask.astype(bool), tril):
        return _numpy_reference(q, k, v, wq, bq, wk, bk, wv, bv, wo, bo, mask)

    in_maps = _prepare_in_maps(q, k, v, wq, bq, wk, bk, wv, bv, wo)
    nc = _get_nc()
    res = run_bass_kernel_spmd(nc, in_maps, core_ids=list(range(NCORES)))

    out = np.empty((B, S, D), np.float32)
    for b in range(B):
        acc = res.results[4 * b]["out_p"].astype(np.float32)
        for c in range(4 * b + 1, 4 * b + 4):
            acc = acc + res.results[c]["out_p"].astype(np.float32)
        out[b] = acc + bo[None, :]
    return out


# revision 8
# speedup vs baseline: 3.7872x; 1.2580x over previous
"""Multi-head attention (B=2, S=4096, D=768, H=12) on 8 Trainium2 cores.

Sharding: core c handles batch c//4 and heads 3*(c%4)..3*(c%4)+2.
Each core computes its 3 heads end-to-end (QKV projection, causal
attention, its partial of the output projection); the host sums the 4
per-batch partials and adds the output bias.

v2: fully software-pipelined single schedule. Projection staging DMAs,
projection matmul chains, and output-projection chains are emitted as
small "filler" units paced between the kb-steps of the attention
query-group loop, so the DMA-bound projection prologue and the PE-bound
output projection hide under the ACT(exp)-bound attention window.
Stage DMAs run two groups ahead; projections one group ahead; output
projection two groups behind (where ACT has surplus). Engine
assignment: ACT runs nothing but Exp (no act-table reloads, table
pre-warmed); diagonal mask multiplies ride the otherwise idle
Pool/gpsimd engine (SBUF-only: GPSIMD cannot touch PSUM); projection
epilogues, reciprocals, normalize multiplies and PSUM->SBUF copies on
DVE; PSUM partition-broadcast via a K=1 ones matmul on PE. Diagonal
score/exp/ctx tiles are trimmed to the causally live query range.
"""

import sys

sys.path.insert(0, "/opt/trn_rl_repo")

import ml_dtypes
import numpy as np

import concourse.bass as bass
import concourse.mybir as mybir
import concourse.tile as tile
from concourse.bass_utils import run_bass_kernel_spmd

B, S, D, H = 2, 4096, 768, 12
DK = D // H          # 64
NCORES = 8
HPC = 3              # heads per core
E = HPC * DK         # 192 = per-core projection width
P = 128
DC = D // P          # 6 contraction chunks of 128
SG = S // 512        # 8 query groups of 512
SC = S // P          # 32 token chunks of 128
F32 = mybir.dt.float32
F32R = mybir.dt.float32r
BF16 = mybir.dt.bfloat16
EXP = mybir.ActivationFunctionType.Exp
BF = ml_dtypes.bfloat16
MUL = mybir.AluOpType.mult
ADD = mybir.AluOpType.add

# (slot, base partition) for each head's Q/K storage
HEAD_LOC = [(0, 0), (0, 64), (1, 0)]


def _split_multi_waits(nc):
    """This walrus build encodes exactly one sync wait per TPB instruction
    and refuses to split multi-wait instructions itself. Rewrite each block
    so extra waits land on same-engine NOPs directly before the owner."""
    k = 0
    for f in nc.m.functions:
        for blk in f.blocks:
            out = []
            changed = False
            for inst in blk.instructions:
                si = inst.sync_info
                if si is not None and len(si.on_wait) > 1:
                    changed = True
                    waits = list(si.on_wait)
                    for w in waits[:-1]:
                        nop = mybir.InstNoOp(name=f"splitw-{k}", ins=[], outs=[])
                        k += 1
                        nop.engine = inst.engine
                        nop.sync_info = mybir.SyncInfo(on_wait=[w], on_update=[])
                        out.append(nop)
                    inst.sync_info = mybir.SyncInfo(
                        on_wait=[waits[-1]], on_update=list(si.on_update)
                    )
                out.append(inst)
            if changed:
                blk.instructions = out


def _build_program(repeat=1, parts="all"):
    nc = bass.Bass("TRN2", target_bir_lowering=False, debug=False)

    qT = nc.declare_dram_parameter("qT", [D, S], BF16, isOutput=False)
    kT = nc.declare_dram_parameter("kT", [D, S], BF16, isOutput=False)
    vT = nc.declare_dram_parameter("vT", [D, S], BF16, isOutput=False)
    wqT = nc.declare_dram_parameter("wqT", [D, E], BF16, isOutput=False)
    wkT = nc.declare_dram_parameter("wkT", [D, E], BF16, isOutput=False)
    wvT = nc.declare_dram_parameter("wvT", [D, E], BF16, isOutput=False)
    woT = nc.declare_dram_parameter("woT", [E, D], BF16, isOutput=False)
    bq = nc.declare_dram_parameter("bq", [P, 2], F32, isOutput=False)  # *0.125
    bk = nc.declare_dram_parameter("bk", [P, 2], F32, isOutput=False)
    bv = nc.declare_dram_parameter("bv", [P, E], F32, isOutput=False)
    maskc = nc.declare_dram_parameter("maskc", [P, 4 * 512], BF16, isOutput=False)
    out_p = nc.declare_dram_parameter("out_p", [S, D], BF16, isOutput=True)

    qT_r = qT[:].rearrange("(dc p) s -> p dc s", p=P)
    kT_r = kT[:].rearrange("(dc p) s -> p dc s", p=P)
    vT_r = vT[:].rearrange("(dc p) s -> p dc s", p=P)

    with tile.TileContext(nc) as tc:
        import contextlib

        with contextlib.ExitStack() as ctx:
            const = ctx.enter_context(tc.tile_pool(name="const", bufs=1))
            persist = ctx.enter_context(tc.tile_pool(name="persist", bufs=1))

            # ---- constants (criticality order: QK weights first, wo last) ----
            wq_sb = const.tile([P, DC, E], BF16)
            nc.sync.dma_start(wq_sb[:], wqT[:].rearrange("(dc p) e -> p dc e", p=P))
            wk_sb = const.tile([P, DC, E], BF16)
            nc.sync.dma_start(wk_sb[:], wkT[:].rearrange("(dc p) e -> p dc e", p=P))
            wv_sb = const.tile([P, DC, E], BF16)
            nc.sync.dma_start(wv_sb[:], wvT[:].rearrange("(dc p) e -> p dc e", p=P))
            bq_sb = const.tile([P, 2], F32)
            nc.sync.dma_start(bq_sb[:], bq[:])
            bk_sb = const.tile([P, 2], F32)
            nc.sync.dma_start(bk_sb[:], bk[:])
            bv_sb = const.tile([P, E], F32)
            nc.sync.dma_start(bv_sb[:], bv[:])
            mask_sb = const.tile([P, 4 * 512], BF16)
            nc.sync.dma_start(mask_sb[:], maskc[:])
            wo_sb = const.tile([64, HPC, D], BF16)
            nc.sync.dma_start(wo_sb[:], woT[:].rearrange("(h p) o -> p h o", p=64))
            onesb = const.tile([P, SC], BF16)
            nc.any.memset(onesb[:], 1.0)
            onesf = const.tile([P, 64], F32)
            nc.any.memset(onesf[:], 1.0)
            ones_sb = const.tile([P, 64], F32R)
            nc.vector.tensor_copy(ones_sb[:], onesf[:])
            # warm the Exp activation table while the staging DMAs run
            warm = const.tile([1, 1], F32)
            nc.scalar.activation(warm[:], onesb[0:1, 0:1], EXP)

            # ---- persistent activations ----
            QT_sb = persist.tile([P, 2, S], BF16)
            KT_sb = persist.tile([P, 2, S], BF16)
            V_sb = persist.tile([P, SC, HPC, 65], BF16)
            ctxT_sb = persist.tile([64, HPC, S], BF16)

            for h in range(HPC):
                nc.vector.tensor_copy(V_sb[:, :, h, 64], onesb[:])

            for _rep in range(repeat):
                with tc.tile_pool(name="stg", bufs=6) as stg_pool, \
                     tc.tile_pool(name="vstg", bufs=3) as vstg_pool, \
                     tc.tile_pool(name="att", bufs=6) as att, \
                     tc.tile_pool(name="att2", bufs=2) as att2, \
                     tc.tile_pool(name="nrm", bufs=4) as nrm, \
                     tc.tile_pool(name="ob", bufs=2) as ob, \
                     tc.tile_pool(name="pjp", bufs=2, space="PSUM") as pjp, \
                     tc.tile_pool(name="vp", bufs=1, space="PSUM") as vp, \
                     tc.tile_pool(name="stp", bufs=2, space="PSUM") as stp, \
                     tc.tile_pool(name="ctxp", bufs=3, space="PSUM") as ctxp:

                    qk_stages = {}   # ("q"|"k", sg) -> staged tile
                    v_stages = {}    # sc4 -> staged tile

                    def stage_qk(which, sg, eng=None):
                        src = qT_r if which == "q" else kT_r
                        stg = stg_pool.tile([P, DC, 512], BF16, tag="stage")
                        (eng or nc.sync).dma_start(
                            stg[:], src[:, :, sg * 512:(sg + 1) * 512]
                        )
                        qk_stages[(which, sg)] = stg

                    def proj_qk(which, sg, ec):
                        # one [em, 512] projection chain + epilogue
                        w_sb, b_sb, dst, scale = (
                            (wq_sb, bq_sb, QT_sb, 0.125)
                            if which == "q"
                            else (wk_sb, bk_sb, KT_sb, 1.0)
                        )
                        stg = qk_stages[(which, sg)]
                        em = P if ec == 0 else 64
                        ps = pjp.tile([P, 512], F32, tag="qk")
                        for dc in range(DC):
                            nc.tensor.matmul(
                                ps[:em, :],
                                w_sb[:, dc, ec * P:ec * P + em],
                                stg[:, dc, :],
                                start=(dc == 0), stop=(dc == DC - 1),
                            )
                        dst_ap = (
                            dst[0:64, 1, sg * 512:(sg + 1) * 512]
                            if ec == 1
                            else dst[:, 0, sg * 512:(sg + 1) * 512]
                        )
                        # GPSIMD cannot read PSUM on hw; DVE has the slack
                        nc.vector.tensor_scalar(
                            dst_ap, ps[:em, :],
                            scale, b_sb[:em, ec:ec + 1],
                            MUL, ADD,
                        )

                    def stage_v(j):
                        vstg = vstg_pool.tile([P, DC, 512], BF16, tag="vstage")
                        nc.sync.dma_start(
                            vstg[:], vT_r[:, :, j * 512:(j + 1) * 512]
                        )
                        v_stages[j] = vstg

                    def proj_v(sc):
                        vstg = v_stages[sc // 4]
                        quart = sc % 4
                        ps = vp.tile([P, 256], F32, tag="v")
                        for dc in range(DC):
                            nc.tensor.matmul(
                                ps[:, 0:E],
                                vstg[:, dc, quart * P:(quart + 1) * P],
                                wv_sb[:, dc, :],
                                start=(dc == 0), stop=(dc == DC - 1),
                            )
                        for h in range(HPC):
                            # tensor_tensor on Pool needs the standard ucode
                            # library, which partition_broadcast displaced;
                            # DVE has the slack.
                            nc.vector.tensor_tensor(
                                V_sb[:, sc, h, 0:64],
                                ps[:, h * 64:(h + 1) * 64],
                                bv_sb[:, h * 64:(h + 1) * 64],
                                ADD,
                            )

                    def emit_attn(g, fillers=(), n_pre=0):
                        # fillers: unit closures paced between kb steps so
                        # the PE queue never has a long proj-only run that
                        # would starve the exp stream. They are emitted
                        # between the exps and ctx matmuls of a kb step:
                        # the first n_pre fillers (prev group's normalize)
                        # must precede this group's first ctx matmul in PE
                        # program order, or the ctx-ring wait deadlocks the
                        # in-order scheduling pass.
                        fillers = list(fillers)
                        filled = 0
                        nkb = 4 * (g + 1)
                        ctx_ps = [
                            ctxp.tile([P, 512], F32, tag="ctx", name=f"ctx{h}")
                            for h in range(HPC)
                        ]
                        for kb in range(nkb):
                            dj = kb - (nkb - 4)  # diagonal index 0..3, else <0
                            qoff = 128 * dj if dj > 0 else 0
                            w = 512 - qoff
                            sts = {}
                            for h in range(HPC):
                                slot, p0 = HEAD_LOC[h]
                                st = stp.tile([P, 512], F32, tag="st",
                                              name=f"st{h}")
                                nc.tensor.matmul(
                                    st[:, 0:w],
                                    KT_sb[p0:p0 + 64, slot,
                                          kb * P:(kb + 1) * P],
                                    QT_sb[p0:p0 + 64, slot,
                                          g * 512 + qoff:(g + 1) * 512],
                                    start=True, stop=True,
                                )
                                sts[h] = st
                            ets = {}
                            for h in range(HPC):
                                et = att.tile([P, 512], BF16, tag="et")
                                if dj >= 0:
                                    # diagonal: exp then 0/1 mask multiply
                                    ete = att2.tile([P, 512], BF16, tag="ete")
                                    nc.scalar.activation(
                                        ete[:, 0:w], sts[h][:, 0:w], EXP
                                    )
                                    # SBUF-only, so it can ride the idle
                                    # Pool engine (GPSIMD cannot read PSUM)
                                    nc.gpsimd.tensor_tensor(
                                        et[:, 0:w], ete[:, 0:w],
                                        mask_sb[:, dj * 512 + qoff:
                                                (dj + 1) * 512],
                                        MUL,
                                    )
                                else:
                                    nc.scalar.activation(
                                        et[:, 0:w], sts[h][:, 0:w], EXP
                                    )
                                ets[h] = et
                            want = max(n_pre, (kb + 3) * len(fillers) // nkb)
                            while filled < min(want, len(fillers)):
                                fillers[filled]()
                                filled += 1
                            for h in range(HPC):
                                nc.tensor.matmul(
                                    ctx_ps[h][0:65, qoff:512],
                                    V_sb[:, kb, h, :],
                                    ets[h][:, 0:w],
                                    start=(kb == 0), stop=(kb == nkb - 1),
                                )
                        while filled < len(fillers):
                            fillers[filled]()
                            filled += 1
                        # reciprocals now: ctx chains just stopped, and the
                        # next group's normalize fillers need them without a
                        # DVE round-trip delay.
                        rcs = []
                        for h in range(HPC):
                            rc = nrm.tile([P, 512], F32R, tag="rc")
                            with nc.allow_low_precision(
                                reason="softmax denominator reciprocal; f32r "
                                "rounding is benign here"
                            ):
                                nc.vector.reciprocal(
                                    rc[64:65, :], ctx_ps[h][64:65, :]
                                )
                            rcs.append(rc)
                        return ctx_ps, rcs

                    def norm_h(g, ctx_ps, rcs, h):
                        # ctxT[h, qg slice] = ctx / denominator; denominator
                        # reciprocal broadcast across partitions 0-63 via a
                        # K=1 matmul with a ones row.
                        rc = rcs[h]
                        bc = stp.tile([P, 512], F32, tag="st")
                        nc.tensor.matmul(
                            bc[0:64, :],
                            ones_sb[64:65, :].bitcast(F32R),
                            rc[64:65, :],
                            start=True, stop=True,
                        )
                        rcb = nrm.tile([64, 512], F32, tag="rcb")
                        nc.vector.tensor_copy(rcb[:], bc[0:64, :])
                        nc.vector.tensor_tensor(
                            ctxT_sb[0:64, h, g * 512:(g + 1) * 512],
                            ctx_ps[h][0:64, :],
                            rcb[:],
                            MUL,
                        )

                    def oproj_sc(sc):
                        osb = ob.tile([P, D], BF16, tag="osb")
                        ps0 = pjp.tile([P, 512], F32, tag="qk")
                        for h in range(HPC):
                            nc.tensor.matmul(
                                ps0[:, :],
                                ctxT_sb[0:64, h, sc * P:(sc + 1) * P],
                                wo_sb[:, h, 0:512],
                                start=(h == 0), stop=(h == HPC - 1),
                            )
                        nc.vector.tensor_copy(osb[:, 0:512], ps0[:, :])
                        ps1 = vp.tile([P, 256], F32, tag="v")
                        for h in range(HPC):
                            nc.tensor.matmul(
                                ps1[:, :],
                                ctxT_sb[0:64, h, sc * P:(sc + 1) * P],
                                wo_sb[:, h, 512:768],
                                start=(h == 0), stop=(h == HPC - 1),
                            )
                        nc.vector.tensor_copy(osb[:, 512:768], ps1[:, :])
                        nc.sync.dma_start(
                            out_p[sc * P:(sc + 1) * P, :], osb[:]
                        )

                    def mk(fn, *a):
                        return lambda: fn(*a)

                    # --- software-pipelined schedule with paced fillers ---
                    # prerequisites for attn group 0, plus the group-1
                    # stages (stage DMAs run two groups ahead so their
                    # latency is fully hidden); q0/k0 staged via the idle
                    # ACT queue so they overlap the const DMAs still
                    # draining on the sync queue.
                    stage_qk("q", 0, eng=nc.scalar)
                    stage_qk("k", 0, eng=nc.scalar)
                    stage_v(0)
                    stage_qk("q", 1)
                    stage_qk("k", 1)
                    stage_v(1)
                    proj_qk("q", 0, 0)
                    proj_qk("q", 0, 1)
                    proj_qk("k", 0, 0)
                    proj_qk("k", 0, 1)
                    for sc in range(4):
                        proj_v(sc)

                    prev = None  # (g, ctx_ps, rcs) of the previous group
                    for g in range(SG):
                        fill = []
                        if prev is not None:
                            # normalize the previous group first (unblocks
                            # its ctx psum ring + its output projection)
                            pg, pctx, prcs = prev
                            for h in range(HPC):
                                fill.append(mk(norm_h, pg, pctx, prcs, h))
                        if g + 2 < SG:
                            # stage DMAs for group g+2 (pure DMA, no PE/ACT)
                            fill.append(mk(stage_qk, "q", g + 2))
                            fill.append(mk(stage_qk, "k", g + 2))
                            fill.append(mk(stage_v, g + 2))
                        # project inputs needed by group g+1 (staged one
                        # group earlier), interleaved with the output
                        # projection of group g-2 (shifted back so its PE
                        # load lands where ACT has surplus; group 6's rides
                        # with 5's in group 7) so no single boundary gets a
                        # burst of PE-only work.
                        gn = g + 1
                        pjs = []
                        if gn < SG:
                            pjs = [
                                mk(proj_qk, "q", gn, 0),
                                mk(proj_qk, "k", gn, 0),
                                mk(proj_v, 4 * gn),
                                mk(proj_qk, "q", gn, 1),
                                mk(proj_qk, "k", gn, 1),
                                mk(proj_v, 4 * gn + 1),
                                mk(proj_v, 4 * gn + 2),
                                mk(proj_v, 4 * gn + 3),
                            ]
                        ops = []
                        opg = []
                        if g >= 2:
                            opg.append(g - 2)
                        if g == SG - 1:
                            opg.append(g - 1)
                        for pg2 in opg:
                            for sc in range(4 * pg2, 4 * pg2 + 4):
                                ops.append(mk(oproj_sc, sc))
                        for i in range(max(len(pjs), len(ops))):
                            if i < len(pjs):
                                fill.append(pjs[i])
                            if i < len(ops):
                                fill.append(ops[i])
                        ctx_ps, rcs = emit_attn(
                            g, fill, n_pre=HPC if prev is not None else 0
                        )
                        prev = (g, ctx_ps, rcs)
                    for h in range(HPC):
                        norm_h(SG - 1, prev[1], prev[2], h)
                    for sc in range(4 * (SG - 1), 4 * SG):
                        oproj_sc(sc)

    _split_multi_waits(nc)
    return nc


_CACHED_NC = None


def _get_nc():
    global _CACHED_NC
    if _CACHED_NC is None:
        _CACHED_NC = _build_program()
    return _CACHED_NC


def _numpy_reference(q, k, v, wq, bq, wk, bk, wv, bv, wo, bo, mask):
    """Fallback for masks the fast path does not handle (non-causal)."""
    out = np.empty((B, S, D), np.float32)
    scale = 1.0 / np.sqrt(DK)
    for b in range(B):
        Q = (q[b] @ wq.T + bq).reshape(S, H, DK).transpose(1, 0, 2)
        K = (k[b] @ wk.T + bk).reshape(S, H, DK).transpose(1, 0, 2)
        V = (v[b] @ wv.T + bv).reshape(S, H, DK).transpose(1, 0, 2)
        ctx = np.empty((H, S, DK), np.float32)
        for h in range(H):
            s = (Q[h] @ K[h].T) * scale
            s = np.where(mask, s, -1e9)
            s -= s.max(axis=-1, keepdims=True)
            e = np.exp(s)
            p = e / e.sum(axis=-1, keepdims=True)
            ctx[h] = p @ V[h]
        out[b] = ctx.transpose(1, 0, 2).reshape(S, D) @ wo.T + bo
    return out


def _prepare_in_maps(q, k, v, wq, bq, wk, bk, wv, bv, wo):
    # causal 0/1 diagonal-block masks: maskc[k, j*512+q] = (128j + k) <= q
    kk = np.arange(P)[:, None]
    qq = np.arange(512)[None, :]
    maskc = np.zeros((P, 4, 512), np.float32)
    for j in range(4):
        maskc[:, j, :] = (P * j + kk) <= qq
    maskc = np.ascontiguousarray(maskc.reshape(P, 4 * 512)).astype(BF)

    wqT = np.ascontiguousarray(wq.T).astype(BF)  # [d_in, e_out]
    wkT = np.ascontiguousarray(wk.T).astype(BF)
    wvT = np.ascontiguousarray(wv.T).astype(BF)
    woT = np.ascontiguousarray(wo.T)             # [e_in, d_out]

    qTb = [np.ascontiguousarray(q[b].T).astype(BF) for b in range(B)]
    kTb = [np.ascontiguousarray(k[b].T).astype(BF) for b in range(B)]
    vTb = [np.ascontiguousarray(v[b].T).astype(BF) for b in range(B)]

    def pack_bias(bvec, scale):
        t = np.zeros((P, 2), np.float32)
        t[:, 0] = bvec[:P] * scale
        t[:64, 1] = bvec[P:E] * scale
        return t

    in_maps = []
    for c in range(NCORES):
        b = c // 4
        e0 = 3 * (c % 4) * DK
        bvp = np.broadcast_to(bv[e0:e0 + E][None, :], (P, E)).copy()
        in_maps.append({
            "qT": qTb[b],
            "kT": kTb[b],
            "vT": vTb[b],
            "wqT": np.ascontiguousarray(wqT[:, e0:e0 + E]),
            "wkT": np.ascontiguousarray(wkT[:, e0:e0 + E]),
            "wvT": np.ascontiguousarray(wvT[:, e0:e0 + E]),
            "woT": np.ascontiguousarray(woT[e0:e0 + E, :]).astype(BF),
            "bq": pack_bias(bq[e0:e0 + E], 0.125),
            "bk": pack_bias(bk[e0:e0 + E], 1.0),
            "bv": bvp,
            "maskc": maskc,
        })
    return in_maps


def kernel(q, k, v, wq, bq, wk, bk, wv, bv, wo, bo, mask, **_unused):
    q = np.asarray(q, np.float32)
    k = np.asarray(k, np.float32)
    v = np.asarray(v, np.float32)
    wq = np.asarray(wq, np.float32)
    wk = np.asarray(wk, np.float32)
    wv = np.asarray(wv, np.float32)
    wo = np.asarray(wo, np.float32)
    bq = np.asarray(bq, np.float32)
    bk = np.asarray(bk, np.float32)
    bv = np.asarray(bv, np.float32)
    bo = np.asarray(bo, np.float32)
    mask = np.asarray(mask)

    tril = np.tril(np.ones((S, S), bool))
    if mask.shape != (S, S) or not np.array_equal(mask.astype(bool), tril):
        return _numpy_reference(q, k, v, wq, bq, wk, bk, wv, bv, wo, bo, mask)

    in_maps = _prepare_in_maps(q, k, v, wq, bq, wk, bk, wv, bv, wo)
    nc = _get_nc()
    res = run_bass_kernel_spmd(nc, in_maps, core_ids=list(range(NCORES)))

    out = np.empty((B, S, D), np.float32)
    for b in range(B):
        acc = res.results[4 * b]["out_p"].astype(np.float32)
        for c in range(4 * b + 1, 4 * b + 4):
            acc = acc + res.results[c]["out_p"].astype(np.float32)
        out[b] = acc + bo[None, :]
    return out
